# revision 27
# baseline (speedup 1.0000x reference)
"""Trainium2 Bass kernel for nn_AELoss (MSE + smooth loss), 8-core data-parallel.

Strategy
--------
Shard batch dim (2048) across 8 cores -> 256 rows/core. Per core, per
(b-group of 128, c) step, DMA-load inputs+targets as ONE [128, 2, 300, 25]
bf16 tile (SWDGE cast f32->bf16 in the DMA, so HBM reads stay f32 but all
on-chip compute runs in bf16 / 2x DVE mode).

Math (per b, c, j):  with A = sum_t x[t], Q = sum_t x[t]^2:
    s_x = A - x[T-1] - Q + x[0]^2   (= sum_{t<T-1} x[t] - x[t+1]^2)
    total[b,c] = sum_{j<J-1} |s_in - s_tgt|
    smooth partial = sum_{b,c} sqrt(total) / (J*T)
    mse partial    = sum x^2 + sum y^2 - 2*sum x*y  (reuses Q sums + one
                     scalar_tensor_tensor pass with accum for the cross term)

Engines: DVE does fold-trees over t (bf16 tensor_tensor at 2x) and the
cross-term pass; ACT does the squares; gpsimd issues cast-DMAs and the final
partition reduction. Per-core partial sums are returned as a [1,2] tensor;
the host combines the 8 cores' partials into the final scalar.
"""

import os
import sys

for _p in ("/opt/trn_rl_repo", "/root/.axon_site"):
    if os.path.isdir(_p) and _p not in sys.path:
        sys.path.insert(0, _p)

import numpy as np

import concourse.bass as bass
import concourse.tile as tile
from concourse import bacc, bass_isa, mybir
from concourse.bass_utils import run_bass_kernel_spmd

N_CORES = 8
B, C, T, J = 2048, 3, 300, 25
B_LOC = B // N_CORES          # 256 batch rows per core
P = 128                       # SBUF partitions
NG = B_LOC // P               # 2 b-groups per core
F32 = mybir.dt.float32
BF16 = mybir.dt.bfloat16
NSTEP = NG * C                # 6 (b-group, c) steps


def _fold_t1(nc, fs_pool, src, res):
    """Sum src [P, 300, 25] over the t axis -> res [P, 25] f32.

    Binary fold tree in bf16: 300 = 2*128 + 44, halve down to 2 rows,
    final add writes f32.
    """
    v = nc.vector
    fs = fs_pool.tile([P, 128, J], BF16, tag="fold_bf")
    v.tensor_add(fs[:, 0:128, :], src[:, 0:128, :], src[:, 128:256, :])
    v.tensor_add(fs[:, 0:44, :], fs[:, 0:44, :], src[:, 256:300, :])
    n = 64
    while n >= 2:
        v.tensor_add(fs[:, 0:n, :], fs[:, 0:n, :], fs[:, n : 2 * n, :])
        n //= 2
    v.tensor_add(res[:, :], fs[:, 0, :], fs[:, 1, :])


def _body(tc, nc, x_d, y_d, out_d):
    sub = mybir.AluOpType.subtract
    add = mybir.AluOpType.add
    mult = mybir.AluOpType.mult
    bypass = mybir.AluOpType.bypass

    TH = T // 2  # 150, t-half for DMA/elementwise pipelining

    with (
        tc.tile_pool(name="inp", bufs=4) as inp_pool,
        tc.tile_pool(name="sd", bufs=3) as sd_pool,
        tc.tile_pool(name="wp", bufs=2) as w_pool,
        tc.tile_pool(name="fold", bufs=2) as fold_pool,
        tc.tile_pool(name="small", bufs=3) as small_pool,
        tc.tile_pool(name="persist", bufs=1) as persist,
    ):
        totals6 = persist.tile([P, NSTEP], F32)       # per-step sum_j |s_in - s_tgt|
        mse14 = persist.tile([P, 2 * NSTEP + 2], F32)  # per-chunk sum (x-y)^2

        k = 0
        mcol = 0
        for g in range(NG):
            for c in range(C):
                # w = -(s-1)*d = d - (x^2-y^2), folded over t for the smooth
                # term; s = x+y and d = x-y live per t-chunk only.
                w = w_pool.tile([P, T, J], BF16, tag="w")
                p0 = small_pool.tile([P, J], BF16, tag="p0")
                d299 = small_pool.tile([P, J], BF16, tag="d299")
                # first step uses quarter chunks so compute starts sooner
                nch = 4 if k == 0 else 2
                tc_sz = T // nch
                for h in range(nch):
                    t0, t1 = h * tc_sz, (h + 1) * tc_sz
                    xyh = inp_pool.tile([P, 2, tc_sz, J], BF16, tag="xy")
                    nc.gpsimd.dma_start(
                        out=xyh[:, 0, :, :],
                        in_=x_d[g * P : (g + 1) * P, c, t0:t1, :],
                    )
                    nc.gpsimd.dma_start(
                        out=xyh[:, 1, :, :],
                        in_=y_d[g * P : (g + 1) * P, c, t0:t1, :],
                    )
                    sdh = sd_pool.tile([P, 2, tc_sz, J], BF16, tag="sd")
                    nc.vector.tensor_add(
                        sdh[:, 0, :, :], xyh[:, 0, :, :], xyh[:, 1, :, :]
                    )
                    nc.vector.tensor_sub(
                        sdh[:, 1, :, :], xyh[:, 0, :, :], xyh[:, 1, :, :]
                    )
                    # w-chunk = (s-1)*d = (x^2-y^2) - (x-y)  [negated w]
                    nc.vector.scalar_tensor_tensor(
                        out=w[:, t0:t1, :],
                        in0=sdh[:, 0, :, :],
                        scalar=1.0,
                        in1=sdh[:, 1, :, :],
                        op0=sub,
                        op1=mult,
                    )
                    # MSE partial for this chunk: sum d^2 (ACT square with
                    # accumulate; junk elementwise output goes to the xyh
                    # tile we just consumed)
                    nc.scalar.activation(
                        xyh[:, 0, :, :],
                        sdh[:, 1, :, :],
                        mybir.ActivationFunctionType.Square,
                        accum_out=mse14[:, mcol : mcol + 1],
                    )
                    mcol += 1
                    if h == 0:
                        # boundary p[0] = s[0]*d[0]
                        nc.vector.tensor_mul(
                            p0[:, :], sdh[:, 0, 0, :], sdh[:, 1, 0, :]
                        )
                    if h == nch - 1:
                        nc.vector.tensor_copy(
                            d299[:, :], sdh[:, 1, tc_sz - 1, :]
                        )

                # fold: F = sum_t (s-1)*d = -W
                F = small_pool.tile([P, J], F32, tag="F")
                _fold_t1(nc, fold_pool, w, F)

                # D[j] = s_in - s_tgt = W + p[0] - d[T-1] = p[0] - F - d[T-1]
                D = small_pool.tile([P, J], F32, tag="D")
                nc.vector.tensor_sub(D[:, :], p0[:, :], F[:, :])
                nc.vector.tensor_sub(D[:, :], D[:, :], d299[:, :])
                nc.vector.reduce_sum(
                    totals6[:, k : k + 1],
                    D[:, 0 : J - 1],
                    axis=mybir.AxisListType.X,
                    apply_absolute_value=True,
                )

                k += 1

        # tail: sqrt(total)/(J*T) == sqrt(total * (1/(J*T))^2), summed over steps
        roots = small_pool.tile([P, NSTEP], F32, tag="roots")
        nc.scalar.activation(
            roots[:, :],
            totals6[:, :],
            mybir.ActivationFunctionType.Sqrt,
            scale=1.0 / float((J * T) ** 2),
        )
        final = small_pool.tile([P, 2], F32, tag="final")
        nc.vector.reduce_sum(final[:, 1:2], roots[:, :], axis=mybir.AxisListType.X)
        nc.vector.reduce_sum(final[:, 0:1], mse14[:, :], axis=mybir.AxisListType.X)

        red = small_pool.tile([P, 2], F32, tag="red")
        nc.gpsimd.partition_all_reduce(
            red[:, :], final[:, :], channels=P, reduce_op=bass_isa.ReduceOp.add
        )
        nc.sync.dma_start(out=out_d[0:1, :], in_=red[0:1, :])


_NC_CACHE = None


def _build():
    global _NC_CACHE
    if _NC_CACHE is not None:
        return _NC_CACHE
    nc = bacc.Bacc("TRN2", target_bir_lowering=False, debug=False, num_devices=N_CORES)
    x_d = nc.dram_tensor("inputs", [B_LOC, C, T, J], F32, kind="ExternalInput")
    y_d = nc.dram_tensor("targets", [B_LOC, C, T, J], F32, kind="ExternalInput")
    out_d = nc.dram_tensor("out", [1, 2], F32, kind="ExternalOutput")
    with tile.TileContext(nc) as tc:
        _body(tc, nc, x_d.ap(), y_d.ap(), out_d.ap())
    nc.compile()
    _NC_CACHE = nc
    return nc


def _run(inputs, targets, trace=False, **kw):
    nc = _build()
    inputs = np.ascontiguousarray(inputs, dtype=np.float32)
    targets = np.ascontiguousarray(targets, dtype=np.float32)
    in_maps = [
        {
            "inputs": inputs[i * B_LOC : (i + 1) * B_LOC],
            "targets": targets[i * B_LOC : (i + 1) * B_LOC],
        }
        for i in range(N_CORES)
    ]
    res = run_bass_kernel_spmd(
        nc, in_maps, core_ids=list(range(N_CORES)), trace=trace, **kw
    )
    mse_sum = 0.0
    smooth_sum = 0.0
    for i in range(N_CORES):
        o = res.results[i]["out"]
        mse_sum += float(o[0, 0])
        smooth_sum += float(o[0, 1])
    value = 2.0 * (mse_sum / (B * C * T * J)) + 3.0 * (smooth_sum / (B * C))
    return np.array(value, dtype=np.float32), res


def kernel(inputs, targets):
    value, _ = _run(inputs, targets)
    return value


# revision 29
# speedup vs baseline: 1.0940x; 1.0940x over previous
"""Trainium2 Bass kernel for nn_AELoss (MSE + smooth loss), 8-core data-parallel.

Strategy
--------
Shard batch dim (2048) across 8 cores -> 256 rows/core. Per core, per
(b-group of 128, c) step, DMA-load inputs+targets as ONE [128, 2, 300, 25]
bf16 tile (SWDGE cast f32->bf16 in the DMA, so HBM reads stay f32 but all
on-chip compute runs in bf16 / 2x DVE mode).

Math (per b, c, j):  with A = sum_t x[t], Q = sum_t x[t]^2:
    s_x = A - x[T-1] - Q + x[0]^2   (= sum_{t<T-1} x[t] - x[t+1]^2)
    total[b,c] = sum_{j<J-1} |s_in - s_tgt|
    smooth partial = sum_{b,c} sqrt(total) / (J*T)
    mse partial    = sum x^2 + sum y^2 - 2*sum x*y  (reuses Q sums + one
                     scalar_tensor_tensor pass with accum for the cross term)

Engines: DVE does fold-trees over t (bf16 tensor_tensor at 2x) and the
cross-term pass; ACT does the squares; gpsimd issues cast-DMAs and the final
partition reduction. Per-core partial sums are returned as a [1,2] tensor;
the host combines the 8 cores' partials into the final scalar.
"""

import os
import sys

for _p in ("/opt/trn_rl_repo", "/root/.axon_site"):
    if os.path.isdir(_p) and _p not in sys.path:
        sys.path.insert(0, _p)

import numpy as np

import concourse.bass as bass
import concourse.tile as tile
from concourse import bacc, bass_isa, mybir
from concourse.bass_utils import run_bass_kernel_spmd

N_CORES = 8
B, C, T, J = 2048, 3, 300, 25
B_LOC = B // N_CORES          # 256 batch rows per core
P = 128                       # SBUF partitions
NG = B_LOC // P               # 2 b-groups per core
F32 = mybir.dt.float32
BF16 = mybir.dt.bfloat16
NSTEP = NG * C                # 6 (b-group, c) steps


def _fold_t2(nc, fs_pool, src, res):
    """Sum src [P, 2, 300, 25] over the t axis -> res [P, 2, 25] f32.

    Binary fold tree in bf16: 300 = 2*128 + 44, halve down to 2 rows,
    final add writes f32.
    """
    v = nc.vector
    fs = fs_pool.tile([P, 2, 128, J], BF16, tag="fold_bf")
    v.tensor_add(fs[:, :, 0:128, :], src[:, :, 0:128, :], src[:, :, 128:256, :])
    v.tensor_add(fs[:, :, 0:44, :], fs[:, :, 0:44, :], src[:, :, 256:300, :])
    n = 64
    while n >= 2:
        v.tensor_add(fs[:, :, 0:n, :], fs[:, :, 0:n, :], fs[:, :, n : 2 * n, :])
        n //= 2
    v.tensor_add(res[:, :, :], fs[:, :, 0, :], fs[:, :, 1, :])


def _body(tc, nc, x_d, y_d, out_d):
    sub = mybir.AluOpType.subtract
    add = mybir.AluOpType.add
    mult = mybir.AluOpType.mult
    bypass = mybir.AluOpType.bypass

    TH = T // 2  # 150, t-half for DMA/elementwise pipelining

    with (
        tc.tile_pool(name="inp", bufs=4) as inp_pool,
        tc.tile_pool(name="sd", bufs=3) as sd_pool,
        tc.tile_pool(name="wp", bufs=2) as w_pool,
        tc.tile_pool(name="fold", bufs=2) as fold_pool,
        tc.tile_pool(name="small", bufs=3) as small_pool,
        tc.tile_pool(name="persist", bufs=1) as persist,
    ):
        totals6 = persist.tile([P, NSTEP], F32)       # per-step sum_j |s_in - s_tgt|
        mse14 = persist.tile([P, 2 * NSTEP + 2], F32)  # per-chunk sum (x-y)^2

        k = 0
        mcol = 0
        for g in range(NG):
            for c in range(C):
                # sd[:,0] = s = x+y (-> becomes p = x^2-y^2), sd[:,1] = d = x-y
                sd = sd_pool.tile([P, 2, T, J], BF16, tag="sd")
                # first step uses quarter chunks so compute starts sooner
                nch = 4 if k == 0 else 2
                tc_sz = T // nch
                for h in range(nch):
                    t0, t1 = h * tc_sz, (h + 1) * tc_sz
                    xyh = inp_pool.tile([P, 2, tc_sz, J], BF16, tag="xy")
                    nc.gpsimd.dma_start(
                        out=xyh[:, 0, :, :],
                        in_=x_d[g * P : (g + 1) * P, c, t0:t1, :],
                    )
                    nc.gpsimd.dma_start(
                        out=xyh[:, 1, :, :],
                        in_=y_d[g * P : (g + 1) * P, c, t0:t1, :],
                    )
                    nc.vector.tensor_add(
                        sd[:, 0, t0:t1, :], xyh[:, 0, :, :], xyh[:, 1, :, :]
                    )
                    nc.vector.tensor_sub(
                        sd[:, 1, t0:t1, :], xyh[:, 0, :, :], xyh[:, 1, :, :]
                    )
                    # p = s*d = x^2-y^2, in place over s
                    nc.vector.tensor_mul(
                        sd[:, 0, t0:t1, :], sd[:, 0, t0:t1, :], sd[:, 1, t0:t1, :]
                    )
                    # MSE partial for this chunk: sum d^2 (ACT square with
                    # accumulate; junk elementwise output goes to the xyh
                    # tile we just consumed)
                    nc.scalar.activation(
                        xyh[:, 0, :, :],
                        sd[:, 1, t0:t1, :],
                        mybir.ActivationFunctionType.Square,
                        accum_out=mse14[:, mcol : mcol + 1],
                    )
                    mcol += 1

                # one combined fold chain: res[:,0]=Pd=sum_t p, res[:,1]=Ad=sum_t d
                res = small_pool.tile([P, 2, J], F32, tag="res")
                _fold_t2(nc, fold_pool, sd, res)

                # D[j] = s_in - s_tgt = Ad - Pd + p[0] - d[T-1]
                D = small_pool.tile([P, J], F32, tag="D")
                nc.vector.tensor_sub(D[:, :], res[:, 1, :], res[:, 0, :])
                nc.vector.tensor_add(D[:, :], D[:, :], sd[:, 0, 0, :])
                nc.vector.tensor_sub(D[:, :], D[:, :], sd[:, 1, T - 1, :])
                nc.vector.reduce_sum(
                    totals6[:, k : k + 1],
                    D[:, 0 : J - 1],
                    axis=mybir.AxisListType.X,
                    apply_absolute_value=True,
                )

                k += 1

        # tail: sqrt(total)/(J*T) == sqrt(total * (1/(J*T))^2), summed over steps
        roots = small_pool.tile([P, NSTEP], F32, tag="roots")
        nc.scalar.activation(
            roots[:, :],
            totals6[:, :],
            mybir.ActivationFunctionType.Sqrt,
            scale=1.0 / float((J * T) ** 2),
        )
        final = small_pool.tile([P, 2], F32, tag="final")
        nc.vector.reduce_sum(final[:, 1:2], roots[:, :], axis=mybir.AxisListType.X)
        nc.vector.reduce_sum(final[:, 0:1], mse14[:, :], axis=mybir.AxisListType.X)

        red = small_pool.tile([P, 2], F32, tag="red")
        nc.gpsimd.partition_all_reduce(
            red[:, :], final[:, :], channels=P, reduce_op=bass_isa.ReduceOp.add
        )
        nc.sync.dma_start(out=out_d[0:1, :], in_=red[0:1, :])


_NC_CACHE = None


def _build():
    global _NC_CACHE
    if _NC_CACHE is not None:
        return _NC_CACHE
    nc = bacc.Bacc("TRN2", target_bir_lowering=False, debug=False, num_devices=N_CORES)
    x_d = nc.dram_tensor("inputs", [B_LOC, C, T, J], F32, kind="ExternalInput")
    y_d = nc.dram_tensor("targets", [B_LOC, C, T, J], F32, kind="ExternalInput")
    out_d = nc.dram_tensor("out", [1, 2], F32, kind="ExternalOutput")
    with tile.TileContext(nc) as tc:
        _body(tc, nc, x_d.ap(), y_d.ap(), out_d.ap())
    nc.compile()
    _NC_CACHE = nc
    return nc


def _run(inputs, targets, trace=False, **kw):
    nc = _build()
    inputs = np.ascontiguousarray(inputs, dtype=np.float32)
    targets = np.ascontiguousarray(targets, dtype=np.float32)
    in_maps = [
        {
            "inputs": inputs[i * B_LOC : (i + 1) * B_LOC],
            "targets": targets[i * B_LOC : (i + 1) * B_LOC],
        }
        for i in range(N_CORES)
    ]
    res = run_bass_kernel_spmd(
        nc, in_maps, core_ids=list(range(N_CORES)), trace=trace, **kw
    )
    mse_sum = 0.0
    smooth_sum = 0.0
    for i in range(N_CORES):
        o = res.results[i]["out"]
        mse_sum += float(o[0, 0])
        smooth_sum += float(o[0, 1])
    value = 2.0 * (mse_sum / (B * C * T * J)) + 3.0 * (smooth_sum / (B * C))
    return np.array(value, dtype=np.float32), res


def kernel(inputs, targets):
    value, _ = _run(inputs, targets)
    return value
